# revision 11
# baseline (speedup 1.0000x reference)
"""Trainium2 Bass kernel for nn_MultiHeadAttention_60971355734022.

Full inputs in, full output out. Sharding: 8 cores = 4 batches x 2 head-groups
(8 heads each). Each core computes its (batch, head-group) slice end-to-end:
  - inputs cast to fp16 on host; q/k/v transposed on-chip by the DMA xbar
    (hardware transpose, 2-byte dtype) straight out of DRAM
  - fp16 projections (fp32 PSUM accumulate) produce qhT/khT in [dh, s]
    layout and vh in [s, p] layout with a ones column per head (softmax
    denominators fall out of the PV matmul for free)
  - causal attention in 512-wide query windows: scores^T = khT.T @ qhT for
    both heads issued back-to-back on distinct PE row groups (concurrent),
    one combined exp per k-block, PV lagging one k-block so the PE never
    waits on ACT; exp on ACT with the 1/sqrt(2048) scale fused; diagonal
    blocks masked with a GPSIMD affine_select
  - normalization reads a DVE copy of the PV accumulator (PSUM bank frees
    early), reciprocal + GPSIMD partition broadcast
  - final projection contracts c^T (already in [p, s] layout) with Wf-slice
  - in the repeat>1 benchmark build, the next repeat's input loads and
    projections are woven chunk-by-chunk into the attention loop so the
    tensor engine fills its ACT-bound gaps with projection matmuls
Host combines: out[b] = core(2b) + core(2b+1) + bf.
"""
import sys

sys.path.insert(0, "/opt/trn_rl_repo")

import math

import numpy as np

import concourse.bacc as bacc
import concourse.bass as bass
import concourse.tile as tile
from concourse import mybir
from concourse.bass_utils import run_bass_kernel_spmd

F32 = mybir.dt.float32
F16 = mybir.dt.float16

S = 2048          # sequence length per batch
D = 1024          # model dim
P = 512           # per-core projection cols (8 heads x 64)
NH = 8            # heads per core
DH = 64           # head dim
NKB = S // 128    # 16 k-blocks
SCALE = 1.0 / math.sqrt(2048.0)  # reference scales by 1/sqrt(MAX_LEN)

EXP = mybir.ActivationFunctionType.Exp


class _Filler:
    """Round-robins pending emit-callbacks across the kb-steps of a block."""

    def __init__(self, chunks, n_steps):
        self.chunks = list(chunks)
        self.n_steps = max(1, n_steps)
        self.acc = 0.0
        self.rate = len(self.chunks) / self.n_steps

    def step(self):
        self.acc += self.rate
        while self.chunks and self.acc >= 1.0:
            self.chunks.pop(0)()
            self.acc -= 1.0

    def drain(self):
        for c in self.chunks:
            c()
        self.chunks = []


def build_core_kernel(repeat=1, debug=False):
    nc = bacc.Bacc()

    qin = nc.dram_tensor("qin", [S, D], F16, kind="ExternalInput")
    kin = nc.dram_tensor("kin", [S, D], F16, kind="ExternalInput")
    vin = nc.dram_tensor("vin", [S, D], F16, kind="ExternalInput")
    wq = nc.dram_tensor("wq", [D, P], F16, kind="ExternalInput")
    wk = nc.dram_tensor("wk", [D, P], F16, kind="ExternalInput")
    wv = nc.dram_tensor("wv", [D, P], F16, kind="ExternalInput")
    wf = nc.dram_tensor("wf", [P, D], F16, kind="ExternalInput")
    bqv = nc.dram_tensor("bqv", [P], F32, kind="ExternalInput")
    bkv = nc.dram_tensor("bkv", [P], F32, kind="ExternalInput")
    bvv = nc.dram_tensor("bvv", [1, P], F32, kind="ExternalInput")
    vones = nc.dram_tensor("vones", [128, NKB, NH, 1], F16, kind="ExternalInput")
    out = nc.dram_tensor("out", [S, D], F32, kind="ExternalOutput")
    if debug:
        dqhT = nc.dram_tensor("dqhT", [128, 4, S], F16, kind="ExternalOutput")
        dkhT = nc.dram_tensor("dkhT", [128, 4, S], F16, kind="ExternalOutput")
        dvhh = nc.dram_tensor("dvhh", [128, NKB, NH, DH + 1], F16,
                              kind="ExternalOutput")
        dcT = nc.dram_tensor("dcT", [128, 4, 2, 1024], F16, kind="ExternalOutput")

    nbuf = 2 if repeat > 1 else 1
    with tile.TileContext(nc) as tc:
        with tc.tile_pool(name="persist", bufs=1) as pp, \
             tc.tile_pool(name="ctp", bufs=1) as ctp:
            # persistent intermediates (double-buffered across repeats in
            # benchmark builds so next-repeat projections weave freely)
            qhT2 = [[pp.tile([128, S], F16, name=f"qhT{b}_{i}", tag=f"qhT{b}_{i}")
                     for i in range(4)] for b in range(nbuf)]
            khT2 = [[pp.tile([128, S], F16, name=f"khT{b}_{i}", tag=f"khT{b}_{i}")
                     for i in range(4)] for b in range(nbuf)]
            vhh2 = [pp.tile([128, NKB, NH, DH + 1], F16, name=f"vhh{i}",
                            tag=f"vhh{i}") for i in range(nbuf)]
            cT = [[ctp.tile([128, 1024], F16, name=f"cT{i}_{p}", tag=f"cT{i}_{p}")
                   for p in range(2)] for i in range(4)]
            wtq = pp.tile([128, 8, P], F16, name="wtq", tag="wtq")
            wtk = pp.tile([128, 8, P], F16, name="wtk", tag="wtk")
            wtv = pp.tile([128, 8, P], F16, name="wtv", tag="wtv")
            wft = pp.tile([128, 4, D], F16, name="wft", tag="wft")
            bq_sb = pp.tile([128, 4], F32, name="bq_sb", tag="bq_sb")
            bk_sb = pp.tile([128, 4], F32, name="bk_sb", tag="bk_sb")
            bv_bc = pp.tile([128, P], F32, name="bv_bc", tag="bv_bc")
            nc.gpsimd.dma_start(out=wtq, in_=wq.rearrange("(db p) c -> p db c", p=128))
            nc.gpsimd.dma_start(out=wtk, in_=wk.rearrange("(db p) c -> p db c", p=128))
            nc.gpsimd.dma_start(out=wtv, in_=wv.rearrange("(db p) c -> p db c", p=128))
            nc.gpsimd.dma_start(out=wft, in_=wf.rearrange("(hp p) c -> p hp c", p=128))
            nc.gpsimd.dma_start(out=bq_sb, in_=bqv.rearrange("(pb p) -> p pb", p=128))
            nc.gpsimd.dma_start(out=bk_sb, in_=bkv.rearrange("(pb p) -> p pb", p=128))
            bv_row = pp.tile([1, P], F32, name="bv_row", tag="bv_row")
            nc.gpsimd.dma_start(out=bv_row, in_=bvv[:, :])
            nc.gpsimd.partition_broadcast(bv_bc, bv_row)
            for vh in vhh2:
                nc.sync.dma_start(out=vh[:, :, :, DH:DH + 1], in_=vones[:, :, :, :])

            with tc.tile_pool(name="xtp", bufs=2) as xtp, \
                 tc.tile_pool(name="pjs", bufs=2, space="PSUM") as pjsp, \
                 tc.tile_pool(name="scs", bufs=2, space="PSUM") as scsp, \
                 tc.tile_pool(name="ops", bufs=1, space="PSUM") as opsp, \
                 tc.tile_pool(name="ptp", bufs=2) as ptp, \
                 tc.tile_pool(name="osb", bufs=1) as osbp, \
                 tc.tile_pool(name="nrm", bufs=1) as nrmp:

                def _load_half(which, rep, hf, xts):
                    xin = {"q": qin, "k": kin, "v": vin}[which]
                    xt = xtp.tile([128, 8, S // 2], F16,
                                  name=f"xt_{which}{rep}{hf}", tag="xt")
                    for db in range(8):
                        nc.sync.dma_start_transpose(
                            xt[:, db, :],
                            xin[1024 * hf:1024 * (hf + 1),
                                128 * db:128 * db + 128])
                    xts.append(xt)

                def _load(which, rep):
                    xts = []
                    _load_half(which, rep, 0, xts)
                    _load_half(which, rep, 1, xts)
                    return xts

                def _qk_chunk(which, xts, pb, sc, buf):
                    dst = qhT2[buf] if which == "q" else khT2[buf]
                    wt = wtq if which == "q" else wtk
                    bias = bq_sb if which == "q" else bk_sb
                    xt, lc = xts[sc // 2], sc % 2
                    pj = pjsp.tile([128, 512], F32,
                                   name=f"pj_{which}{pb}{sc}", tag="pj")
                    for db in range(8):
                        nc.tensor.matmul(
                            pj[:, :], wt[:, db, 128 * pb:128 * pb + 128],
                            xt[:, db, 512 * lc:512 * (lc + 1)],
                            start=(db == 0), stop=(db == 7))
                    nc.vector.tensor_scalar_add(
                        dst[pb][:, 512 * sc:512 * (sc + 1)],
                        pj[:, :], bias[:, pb:pb + 1])

                def _v_chunk(xts, vh, sg):
                    xt, ls = xts[sg // 8], sg % 8
                    pj = pjsp.tile([128, 512], F32, name=f"pj_v{sg}", tag="pj")
                    for db in range(8):
                        nc.tensor.matmul(
                            pj[:, :], xt[:, db, 128 * ls:128 * ls + 128],
                            wtv[:, db, :], start=(db == 0), stop=(db == 7))
                    nc.vector.scalar_tensor_tensor(
                        vh[:, sg, :, 0:DH],
                        pj.rearrange("p (h d) -> p h d", h=NH),
                        1.0,
                        bv_bc.rearrange("p (h d) -> p h d", h=NH),
                        mybir.AluOpType.mult,
                        mybir.AluOpType.add)

                def _emit_a_dense(rep):
                    vh = vhh2[rep % nbuf]
                    buf = rep % nbuf
                    xt = _load("v", rep)
                    for sg in range(16):
                        _v_chunk(xt, vh, sg)
                    xt = _load("q", rep)
                    for pb in range(4):
                        for sc in range(4):
                            _qk_chunk("q", xt, pb, sc, buf)
                    xt = _load("k", rep)
                    for pb in range(4):
                        for sc in range(4):
                            _qk_chunk("k", xt, pb, sc, buf)

                def _emit_b_block(rep, hp, filler):
                    vh = vhh2[rep % nbuf]
                    qhT = qhT2[rep % nbuf]
                    khT = khT2[rep % nbuf]
                    for w in range(4):
                        qlo = 512 * w
                        nkb = 4 * w + 4
                        opsum = [opsp.tile([DH + 1, 512], F32,
                                           name=f"op{hp}{w}{h}", tag=f"op{h}")
                                 for h in range(2)]
                        pts = [None] * nkb

                        def _emit_pv(kb):
                            o0 = max(0, 128 * kb - qlo)
                            for h in range(2):
                                nc.tensor.matmul(
                                    opsum[h][:, o0:512],
                                    vh[:, kb, 2 * hp + h, :],
                                    pts[kb][:, 512 * h + o0:512 * h + 512],
                                    start=(kb == 0), stop=(kb == nkb - 1))

                        for kb in range(nkb):
                            o0 = max(0, 128 * kb - qlo)
                            sp = scsp.tile([128, 1024], F32,
                                           name=f"sp{hp}{w}{kb}", tag="sp")
                            for h in range(2):
                                nc.tensor.matmul(
                                    sp[:, 512 * h + o0:512 * h + 512],
                                    khT[hp][64 * h:64 * h + 64,
                                            128 * kb:128 * kb + 128],
                                    qhT[hp][64 * h:64 * h + 64,
                                            qlo + o0:qlo + 512],
                                    start=True, stop=True,
                                    tile_position=(64 * h, 0))
                            pt = ptp.tile([128, 1024], F16,
                                          name=f"pt{hp}{w}{kb}", tag="pt")
                            pts[kb] = pt
                            if o0 == 0:
                                nc.scalar.activation(pt, sp, EXP, scale=SCALE)
                            else:
                                for h in range(2):
                                    nc.scalar.activation(
                                        pt[:, 512 * h + o0:512 * h + 512],
                                        sp[:, 512 * h + o0:512 * h + 512],
                                        EXP, scale=SCALE)
                            if 128 * kb >= qlo:
                                for h in range(2):
                                    nc.gpsimd.affine_select(
                                        pt[:, 512 * h + o0:512 * h + o0 + 128],
                                        pt[:, 512 * h + o0:512 * h + o0 + 128],
                                        pattern=[[1, 128]],
                                        compare_op=mybir.AluOpType.is_ge,
                                        fill=0.0, base=0,
                                        channel_multiplier=-1)
                            if kb >= 1:
                                _emit_pv(kb - 1)
                            filler.step()
                        _emit_pv(nkb - 1)

                        ps, qc = w // 2, w % 2
                        for h in range(2):
                            osb = osbp.tile([DH + 1, 512], F32,
                                            name=f"ob{hp}{w}{h}", tag=f"ob{h}")
                            nc.vector.tensor_copy(osb, opsum[h])
                            rec = nrmp.tile([1, 512], F32,
                                            name=f"rc{hp}{w}{h}", tag="rc")
                            nc.vector.reciprocal(rec, osb[DH:DH + 1, :])
                            rbc = nrmp.tile([64, 512], F32,
                                            name=f"rb{hp}{w}{h}", tag="rb")
                            nc.gpsimd.partition_broadcast(rbc, rec)
                            nc.vector.tensor_mul(
                                cT[hp][ps][64 * h:64 * h + 64,
                                           512 * qc:512 * qc + 512],
                                osb[0:DH, :], rbc)

                def _emit_c(rep):
                    for sb in range(16):
                        for dm in range(2):
                            fp = pjsp.tile([128, 512], F32,
                                           name=f"fp{sb}{dm}", tag="pj")
                            for hp in range(4):
                                nc.tensor.matmul(
                                    fp[:, :],
                                    cT[hp][sb // 8][:, 128 * (sb % 8):
                                                    128 * (sb % 8) + 128],
                                    wft[:, hp, 512 * dm:512 * dm + 512],
                                    start=(hp == 0), stop=(hp == 3))
                            osg = osbp.tile([128, 512], F32,
                                            name=f"os{sb}{dm}", tag="os",
                                            bufs=2)
                            nc.vector.tensor_copy(osg, fp[:, :])
                            nc.gpsimd.dma_start(
                                out=out[128 * sb:128 * sb + 128,
                                        512 * dm:512 * dm + 512],
                                in_=osg)

                # ---------------- schedule ----------------
                # Weave A(rep+1) into B(rep): v-proj chunks fill block hp=0,
                # q-proj block hp=1, k-proj block hp=2; each tensor's two
                # half-loads are emitted as chunks once the previous tensor's
                # slot readers are already queued, so xt slots (shared tag,
                # 2 bufs) free in strict block order.
                _emit_a_dense(0)
                for rep in range(repeat):
                    nxt = rep + 1 if rep + 1 < repeat else None
                    blocks = {0: [], 1: [], 2: [], 3: []}
                    if nxt is not None:
                        nbf = nxt % nbuf
                        vh_n = vhh2[nbf]
                        xt_v, xt_q, xt_k = [], [], []
                        _load_half("v", nxt, 0, xt_v)
                        _load_half("v", nxt, 1, xt_v)
                        b0 = [(lambda sg=sg: _v_chunk(xt_v, vh_n, sg))
                              for sg in range(16)]
                        b0.insert(4, lambda: _load_half("q", nxt, 0, xt_q))
                        b0.insert(13, lambda: _load_half("q", nxt, 1, xt_q))
                        blocks[0] = b0
                        b1 = [(lambda pb=pb, sc=sc:
                               _qk_chunk("q", xt_q, pb, sc, nbf))
                              for sc in range(4) for pb in range(4)]
                        b1.insert(3, lambda: _load_half("k", nxt, 0, xt_k))
                        b1.insert(12, lambda: _load_half("k", nxt, 1, xt_k))
                        blocks[1] = b1
                        blocks[2] = [(lambda pb=pb, sc=sc:
                                      _qk_chunk("k", xt_k, pb, sc, nbf))
                                     for sc in range(4) for pb in range(4)]
                    for hp in range(4):
                        filler = _Filler(blocks[hp], 40)
                        _emit_b_block(rep, hp, filler)
                        filler.drain()
                    _emit_c(rep)

                if debug:
                    for i in range(4):
                        nc.gpsimd.dma_start(out=dqhT[:, i, :], in_=qhT2[0][i])
                        nc.gpsimd.dma_start(out=dkhT[:, i, :], in_=khT2[0][i])
                        for p_ in range(2):
                            nc.gpsimd.dma_start(out=dcT[:, i, p_, :], in_=cT[i][p_])
                    nc.gpsimd.dma_start(out=dvhh[:, :, :, :], in_=vhh2[0])
    nc.finalize()
    return nc


_NC_CACHE = None


def _get_nc():
    global _NC_CACHE
    if _NC_CACHE is None:
        _NC_CACHE = build_core_kernel()
    return _NC_CACHE


def kernel(q, k, v, Wq, bq, Wk, bk, Wv, bv, Wf, bf, trace=False, tmpdir=None):
    q16 = np.asarray(q, np.float32).astype(np.float16)
    k16 = np.asarray(k, np.float32).astype(np.float16)
    v16 = np.asarray(v, np.float32).astype(np.float16)
    Wq16 = np.asarray(Wq, np.float32).astype(np.float16)
    Wk16 = np.asarray(Wk, np.float32).astype(np.float16)
    Wv16 = np.asarray(Wv, np.float32).astype(np.float16)
    Wf16 = np.asarray(Wf, np.float32).astype(np.float16)
    bq = np.asarray(bq, np.float32)
    bk = np.asarray(bk, np.float32)
    bv = np.asarray(bv, np.float32)
    bf = np.asarray(bf, np.float32)

    vones = np.ones((128, NKB, NH, 1), np.float16)

    in_maps = []
    for c in range(8):
        b, g = c // 2, c % 2
        sl = slice(P * g, P * (g + 1))
        in_maps.append({
            "qin": np.ascontiguousarray(q16[b]),
            "kin": np.ascontiguousarray(k16[b]),
            "vin": np.ascontiguousarray(v16[b]),
            "wq": np.ascontiguousarray(Wq16[:, sl]),
            "wk": np.ascontiguousarray(Wk16[:, sl]),
            "wv": np.ascontiguousarray(Wv16[:, sl]),
            "wf": np.ascontiguousarray(Wf16[sl, :]),
            "bqv": np.ascontiguousarray(bq[sl]),
            "bkv": np.ascontiguousarray(bk[sl]),
            "bvv": np.ascontiguousarray(bv[sl])[None, :],
            "vones": vones,
        })

    nc = _get_nc()
    kw = {}
    if trace:
        kw = {"trace": True, "tmpdir": tmpdir}
    res = run_bass_kernel_spmd(nc, in_maps, core_ids=list(range(8)), **kw)

    outp = np.empty((4, S, D), np.float32)
    for b in range(4):
        outp[b] = res.results[2 * b]["out"] + res.results[2 * b + 1]["out"] + bf
    if trace:
        return outp, res
    return outp
